# revision 48
# baseline (speedup 1.0000x reference)
"""Trainium2 Bass kernel for Conv2D(sum of 20 1x1 convs) + QwenRMSNorm.

Math: y = einsum("bsi,loi->bso", x, conv_w) / L ; out = rmsnorm(y) * norm_w.
Since x does not depend on l, the 20-matrix contraction collapses to a single
matmul with W = sum_l conv_w[l] / L.  Host pre-sums/transposes/casts the weight
(one [H,H] matrix) and lays out x as token-sharded, hidden-major bf16 slabs;
the 8 NeuronCores each run matmul (bf16, fp32 accum) + RMSNorm on their 2048
tokens.  All device compute is token-local; no collectives.

Scheduling notes:
  * Per-core HBM share is ~360 GB/s and both HWDGE rings (SP=sync,
    Act=scalar) split it, so the startup critical path is "first w chunk +
    first x chunk".  w streams in 8 fine chunks in exact consumption order
    on the scalar ring; the bulk x prefetch is queued BEHIND w on the same
    ring (FIFO) so it cannot steal bandwidth from the weight load.  Only
    x[tt0,tt1] + norm_w ride the sync ring early.
  * Output is bf16 (upcast on host): halves the output traffic and the
    end-of-kernel DMA drain.  Output DMAs ride the sync ring, which is idle
    after the first microseconds.
  * PE warm-up: the HAM clock gate keeps the PE at 1.2 GHz until it has been
    busy ~3.4us without gaps.  Dummy matmuls bridge the first-DMA wait.
  * x lives in one big SBUF tile; Tile's region tracker gives per-slice
    dependencies so matmuls only gate on the DMA stage that carries their
    slab.
  * While the weights stream (HBM-bound), the first blocks run
    w-chunk-major so each arriving chunk feeds several matmuls, and the
    chunk-arrival gaps in the oh1 half are filled with tt2's oh0 matmuls
    against already-resident chunks - the PE never idles long enough for
    the clock gate to drop.
"""

import numpy as np
import ml_dtypes
from contextlib import ExitStack

import concourse.bass as bass
import concourse.mybir as mybir
import concourse.tile as tile
from concourse.bass_utils import run_bass_kernel_spmd

N_CORES = 8
B, S, H, L = 4, 4096, 1024, 20
TOK = B * S               # 16384 tokens
TPC = TOK // N_CORES      # 2048 tokens per core
TB = TPC // 128           # 16 token-blocks of 128 per core
KB = H // 128             # 8 contraction blocks
NOH = H // 512            # 2 psum halves of the output row
XSL = KB * 128            # 1024 elements per (partition, token-block) of x
EPS = 1e-6

BF16 = mybir.dt.bfloat16
F32 = mybir.dt.float32
AF = mybir.ActivationFunctionType
OP = mybir.AluOpType

_BUILT = None       # cached Bass program
LAST_RESULTS = None  # BassKernelResults of the most recent run (for test harness)


def _legalize_multiwait(nc):
    """The walrus build here encodes exactly one semaphore wait per 64B
    instruction (NEURON_ISA_TPB_EVENTS has a single wait slot) and errors on
    Tile's multi-wait instructions.  Split surplus waits into standalone
    EVENT_SEMAPHORE instructions on the same engine, placed directly before
    the original instruction (same sequencer stream -> same semantics)."""
    n_ev = 0
    for f in nc.m.functions:
        for blk in f.blocks:
            insts = blk.instructions
            out = []
            changed = False
            for inst in list(insts):
                si = getattr(inst, "sync_info", None)
                waits = list(si.on_wait) if si is not None else []
                if len(waits) > 1:
                    changed = True
                    updates = list(si.on_update)
                    for w in waits[:-1]:
                        ev = mybir.InstEventSemaphore(
                            name=f"{inst.name}-sw{n_ev}", ins=[], outs=[])
                        n_ev += 1
                        ev.engine = inst.engine
                        ev.sync_info = mybir.SyncInfo(on_wait=[w], on_update=[])
                        out.append(ev)
                    inst.sync_info = mybir.SyncInfo(
                        on_wait=[waits[-1]], on_update=updates)
                out.append(inst)
            if changed:
                insts.clear()
                insts.extend(out)


def _build(ones_nw=False):
    nc = bass.Bass()
    # x^T slab layout per core: xt[tt, p, ib, t] = x[tt*128 + t, ib*128 + p], bf16
    xt_h = nc.dram_tensor("xt", [TB, 128, KB, 128], BF16, kind="ExternalInput")
    # weight layout: wt[p, ib, o] = W[o, ib*128 + p] with W = sum_l conv_w[l]/L, bf16
    wt_h = nc.dram_tensor("wt", [128, KB, H], BF16, kind="ExternalInput")
    nw_h = nc.dram_tensor("nw", [H], F32, kind="ExternalInput")
    out_h = nc.dram_tensor("out", [TPC, H], BF16, kind="ExternalOutput")

    with tile.TileContext(nc) as tc, ExitStack() as ctx:
        xpool = ctx.enter_context(tc.tile_pool(name="x", bufs=1))
        wpool = ctx.enter_context(tc.tile_pool(name="w", bufs=1))
        cpool = ctx.enter_context(tc.tile_pool(name="consts", bufs=1))
        # out DMA acks lag ~2-3us behind their data; 6 bufs keep the o_sb
        # WAR (tile n vs tile n-6's DMA completion) off the critical path.
        opool = ctx.enter_context(tc.tile_pool(name="out", bufs=6))
        spool = ctx.enter_context(tc.tile_pool(name="scratch", bufs=2))
        stats = ctx.enter_context(tc.tile_pool(name="stats", bufs=8))
        psum = ctx.enter_context(tc.tile_pool(name="psum", bufs=4, space="PSUM"))

        w_sb = wpool.tile([128, KB, H], BF16)
        x_sb = xpool.tile([128, TB, KB, 128], BF16)
        G = 2  # token blocks computed w-chunk-major while the weights stream

        def x_dma(eng, a, b, ib0=0, ib1=KB):
            # x slabs [a, b) x contraction blocks [ib0, ib1) -> x_sb slice
            src = bass.AP(tensor=xt_h, offset=a * 128 * XSL + ib0 * 128,
                          ap=[[XSL, 128], [128 * XSL, b - a],
                              [1, (ib1 - ib0) * 128]])
            eng.dma_start(out=x_sb[:, a:b, ib0:ib1, :], in_=src)

        # Weight chunks are FULL-H per-ib rows: 2KB contiguous per
        # partition -> 2KB DMA descriptor lines (~2x the early-phase
        # throughput of 1KB oh-half slices), alternating across both
        # HWDGE rings so arrivals interleave.  A chunk serves BOTH oh
        # halves, so the whole weight is resident when the oh0 pass ends
        # and the oh1 pass runs wait-free.  Only ib0 is split by oh half
        # so the very first matmul gates on just 128KB.  Bulk x rides
        # strictly behind the weights.  norm_w broadcast is on gpsimd.
        def w_dma(eng, ib0, ib1, c0=0, c1=H):
            eng.dma_start(out=w_sb[:, ib0:ib1, c0:c1],
                          in_=wt_h[:, ib0:ib1, c0:c1])

        # sync ring: xA, ib2, xB, ib4, ib6, x8-11 (then output DMAs)
        x_dma(nc.sync, 0, G, 0, KB // 2)
        w_dma(nc.sync, 2, 3)
        x_dma(nc.sync, 0, G, KB // 2, KB)
        w_dma(nc.sync, 4, 5)
        w_dma(nc.sync, 6, 7)
        x_dma(nc.sync, 8, 12)
        # scalar ring: ib0-oh0, ib1, ib3, ib5, ib7, ib0-oh1, x2, x3, bulk
        w_dma(nc.scalar, 0, 1, 0, 512)
        w_dma(nc.scalar, 1, 2)
        w_dma(nc.scalar, 3, 4)
        w_dma(nc.scalar, 5, 6)
        w_dma(nc.scalar, 7, 8)
        w_dma(nc.scalar, 0, 1, 512, H)
        x_dma(nc.scalar, 2, 3)     # x2
        x_dma(nc.scalar, 3, 4)     # x3
        x_dma(nc.scalar, 4, 8)
        x_dma(nc.scalar, 12, 16)

        # norm_w broadcast rides the gpsimd (SWDGE) ring, off the two HWDGE
        # rings that carry the latency-critical w/x stream.
        nw_sb = cpool.tile([128, H], F32)
        nc.gpsimd.dma_start(
            out=nw_sb, in_=bass.AP(tensor=nw_h, offset=0, ap=[[0, 128], [1, H]]))

        zero_sb = cpool.tile([128, 1], F32)
        nc.vector.memset(zero_sb, 0.0)
        eps_sb = cpool.tile([128, 1], F32)
        nc.vector.memset(eps_sb, EPS)

        # PE warm-up: dummy matmuls bridge the gap until the first w/x
        # chunks land, so the HAM clock gate sees uninterrupted activity
        # from well before the first real matmul and ramps the PE to
        # 2.4 GHz ~3.4us after the stream starts.  They write regions of
        # psum that later blocks overwrite (start=True clears the bank), so
        # no extra PSUM bank is needed.
        dummy = cpool.tile([128, 128], BF16)
        nc.vector.memset(dummy, 0.0)
        yps = [psum.tile([128, H], F32, name="yp", tag="yp") for _ in range(G)]
        # tt2's psum tile doubles as the dummy-matmul target: its real
        # accumulation group only opens in phase 1b (start=True clears the
        # bank), so dummies can write it any time before that - unlike
        # yps[0/1], whose groups are OPEN during phase 1a (a dummy's
        # start=True there would clear live partial sums).
        yp2 = psum.tile([128, H], F32, name="yp", tag="yp")

        def warm(n):
            for _ in range(n):
                nc.tensor.matmul(yp2[:, 0:128], dummy, dummy,
                                 start=True, stop=True)

        warm(34)

        sq = spool.tile([128, H], BF16)  # squares scratch, shared (write-only)

        def mk_square(yp, oh, half_sums):
            sl = slice(oh * 512, (oh + 1) * 512)
            nc.scalar.activation(out=sq[:, sl], in_=yp[:, sl],
                                 func=AF.Square, bias=zero_sb,
                                 accum_out=half_sums[:, oh:oh + 1])

        def finish_norm(tt, yp, half_sums, last=False, ones_nw=False):
            ssum = stats.tile([128, 1], F32)
            nc.vector.tensor_add(out=ssum, in0=half_sums[:, 0:1],
                                 in1=half_sums[:, 1:2])
            # std = sqrt(mean + eps); rstd = 1/std
            std = stats.tile([128, 1], F32)
            nc.scalar.activation(out=std, in_=ssum, func=AF.Sqrt,
                                 bias=eps_sb, scale=1.0 / H)
            rstd = stats.tile([128, 1], F32)
            nc.vector.reciprocal(out=rstd, in_=std)
            # out = (y * rstd) * norm_w, written bf16.  The last block is
            # split finer so its DMA starts as soon as possible.
            if last and ones_nw:
                # norm_w is all-ones: split the final scale across BOTH
                # engines so the two halves run in parallel (ACT's free
                # per-partition scale does y*rstd directly), and the two
                # half DMAs across two rings so the triggers overlap.
                # TWO dedicated, never-reused half tiles: pooled tiles'
                # WAR deps run through 8 shared DMA semaphore lanes (false
                # serialization on an old DMA ack), and a single shared
                # tile WAW-serializes the two engines' writes.
                o_h0 = opool.tile([128, 512], BF16, name="o_last0",
                                  tag="olast0", bufs=1)
                o_h1 = opool.tile([128, 512], BF16, name="o_last1",
                                  tag="olast1", bufs=1)
                nc.scalar.activation(out=o_h1, in_=yp[:, 512:1024],
                                     func=AF.Copy, scale=rstd)
                nc.vector.scalar_tensor_tensor(
                    out=o_h0, in0=yp[:, 0:512], scalar=rstd,
                    in1=nw_sb[:, 0:512], op0=OP.mult, op1=OP.mult,
                )
                nc.sync.dma_start(out=out_h[tt * 128:(tt + 1) * 128, 0:512],
                                  in_=o_h0)
                nc.gpsimd.dma_start(
                    out=out_h[tt * 128:(tt + 1) * 128, 512:1024],
                    in_=o_h1)
                return
            if last:
                o_sb = opool.tile([128, H], BF16, name="o_last",
                                  tag="olast", bufs=1)
            else:
                o_sb = opool.tile([128, H], BF16, name="o_sb", tag="o")
            for q in range(2):
                sl = slice(q * 512, (q + 1) * 512)
                nc.vector.scalar_tensor_tensor(
                    out=o_sb[:, sl], in0=yp[:, sl], scalar=rstd,
                    in1=nw_sb[:, sl], op0=OP.mult, op1=OP.mult,
                )
                if last:
                    nc.sync.dma_start(
                        out=out_h[tt * 128:(tt + 1) * 128, sl],
                        in_=o_sb[:, sl])
            if not last:
                nc.sync.dma_start(out=out_h[tt * 128:(tt + 1) * 128, :],
                                  in_=o_sb)

        # Phase 1: w-chunk-major over the first G token blocks.  Each
        # arriving 256KB w chunk feeds 2*G matmuls, so the PE's weight
        # demand drops to what HBM can deliver while 8 cores all pull their
        # weights.  Dummy matmuls pad the arrival gaps so the HAM clock
        # gate sees continuous activity and ramps to 2.4 GHz early.
        hs = {}
        # Phase 1a: oh0 chunks c1..c4, chunk-major over t0,t1.  ib-major so
        # a late-arriving ib never head-of-line-blocks matmuls whose data
        # is already present; the c1a->c1b arrival gap (~2us, the fabric is
        # still ramping) is bridged with dummies so the clock gate holds.
        for ibp in range(0, KB, 2):
            for ib in (ibp, ibp + 1):
                for t in range(G):
                    nc.tensor.matmul(
                        yps[t][:, 0:512],
                        x_sb[:, t, ib, :],
                        w_sb[:, ib, 0:512],
                        start=(ib == 0),
                        stop=(ib == KB - 1),
                    )
                if ib == 0:
                    warm(20)
        for t in range(G):
            hs[t] = stats.tile([128, 2], F32, name="hs", tag="hs")
            mk_square(yps[t], 0, hs[t])
        # Phase 1b: the oh1 pass over t0,t1 - the per-ib weight chunks are
        # all resident by now, so this runs wait-free at full clock.
        for ib in range(KB):
            for t in range(G):
                nc.tensor.matmul(
                    yps[t][:, 512:1024],
                    x_sb[:, t, ib, :],
                    w_sb[:, ib, 512:1024],
                    start=(ib == 0),
                    stop=(ib == KB - 1),
                )
        for t in range(G):
            mk_square(yps[t], 1, hs[t])
        for t in range(G):
            finish_norm(t, yps[t], hs[t])

        # Phase 2: weights are resident; token-block-major.  tt2 reuses
        # yp2 (the dummy-matmul target; its group opens here).
        for tt in range(G, TB):
            yp = yp2 if tt == G else psum.tile([128, H], F32,
                                               name="yp", tag="yp")
            half_sums = stats.tile([128, 2], F32, name="hs", tag="hs")
            for oh in range(NOH):
                for ib in range(KB):
                    nc.tensor.matmul(
                        yp[:, oh * 512:(oh + 1) * 512],
                        x_sb[:, tt, ib, :],
                        w_sb[:, ib, oh * 512:(oh + 1) * 512],
                        start=(ib == 0),
                        stop=(ib == KB - 1),
                    )
                mk_square(yp, oh, half_sums)
            finish_norm(tt, yp, half_sums, last=(tt == TB - 1),
                        ones_nw=ones_nw)

    _legalize_multiwait(nc)
    return nc


def host_prep(x, conv_w, norm_w):
    """Shard + lay out the full inputs into per-core device input maps."""
    bf16 = ml_dtypes.bfloat16

    # Collapse the 20 1x1 convs: W[o,i] = sum_l conv_w[l,o,i] / L
    w = np.asarray(conv_w).sum(axis=0) * (1.0 / L)          # [H(o), H(i)] f32
    # wt[p, ib, o] = W[o, ib*128+p]
    wt = np.ascontiguousarray(
        w.reshape(H, KB, 128).transpose(2, 1, 0).astype(bf16))
    nw = np.ascontiguousarray(np.asarray(norm_w), dtype=np.float32)

    x2d = np.asarray(x).reshape(TOK, H)
    xbf = x2d.astype(bf16)

    in_maps = []
    for c in range(N_CORES):
        xc = xbf[c * TPC:(c + 1) * TPC]                      # [TPC, H]
        # xt[tt, p, ib, t] = xc[tt*128+t, ib*128+p]
        xtc = np.ascontiguousarray(
            xc.reshape(TB, 128, KB, 128).transpose(0, 3, 2, 1))
        in_maps.append({"xt": xtc, "wt": wt, "nw": nw})
    return in_maps


def kernel(x, conv_w, norm_w):
    global _BUILT, LAST_RESULTS
    ones_nw = bool(np.allclose(np.asarray(norm_w), 1.0))
    if _BUILT is None:
        _BUILT = {}
    if ones_nw not in _BUILT:
        _BUILT[ones_nw] = _build(ones_nw)
    nc = _BUILT[ones_nw]

    x = np.asarray(x)
    out_dtype = x.dtype
    in_maps = host_prep(x, conv_w, norm_w)

    res = run_bass_kernel_spmd(nc, in_maps, core_ids=list(range(N_CORES)))
    LAST_RESULTS = res

    out = np.concatenate([r["out"] for r in res.results], axis=0)
    return out.reshape(B, S, H).astype(out_dtype, copy=False)


# revision 52
# speedup vs baseline: 1.0324x; 1.0324x over previous
"""Trainium2 Bass kernel for Conv2D(sum of 20 1x1 convs) + QwenRMSNorm.

Math: y = einsum("bsi,loi->bso", x, conv_w) / L ; out = rmsnorm(y) * norm_w.
Since x does not depend on l, the 20-matrix contraction collapses to a single
matmul with W = sum_l conv_w[l] / L.  Host pre-sums/transposes/casts the weight
(one [H,H] matrix) and lays out x as token-sharded, hidden-major bf16 slabs;
the 8 NeuronCores each run matmul (bf16, fp32 accum) + RMSNorm on their 2048
tokens.  All device compute is token-local; no collectives.

Scheduling notes:
  * Per-core HBM share is ~360 GB/s and both HWDGE rings (SP=sync,
    Act=scalar) split it, so the startup critical path is "first w chunk +
    first x chunk".  w streams in 8 fine chunks in exact consumption order
    on the scalar ring; the bulk x prefetch is queued BEHIND w on the same
    ring (FIFO) so it cannot steal bandwidth from the weight load.  Only
    x[tt0,tt1] + norm_w ride the sync ring early.
  * Output is bf16 (upcast on host): halves the output traffic and the
    end-of-kernel DMA drain.  Output DMAs ride the sync ring, which is idle
    after the first microseconds.
  * PE warm-up: the HAM clock gate keeps the PE at 1.2 GHz until it has been
    busy ~3.4us without gaps.  Dummy matmuls bridge the first-DMA wait.
  * x lives in one big SBUF tile; Tile's region tracker gives per-slice
    dependencies so matmuls only gate on the DMA stage that carries their
    slab.
  * While the weights stream (HBM-bound), the first blocks run
    w-chunk-major so each arriving chunk feeds several matmuls, and the
    chunk-arrival gaps in the oh1 half are filled with tt2's oh0 matmuls
    against already-resident chunks - the PE never idles long enough for
    the clock gate to drop.
"""

import numpy as np
import ml_dtypes
from contextlib import ExitStack

import concourse.bass as bass
import concourse.mybir as mybir
import concourse.tile as tile
from concourse.bass_utils import run_bass_kernel_spmd

N_CORES = 8
B, S, H, L = 4, 4096, 1024, 20
TOK = B * S               # 16384 tokens
TPC = TOK // N_CORES      # 2048 tokens per core
TB = TPC // 128           # 16 token-blocks of 128 per core
KB = H // 128             # 8 contraction blocks
NOH = H // 512            # 2 psum halves of the output row
XSL = KB * 128            # 1024 elements per (partition, token-block) of x
EPS = 1e-6

BF16 = mybir.dt.bfloat16
F32 = mybir.dt.float32
AF = mybir.ActivationFunctionType
OP = mybir.AluOpType

_BUILT = None       # cached Bass program
LAST_RESULTS = None  # BassKernelResults of the most recent run (for test harness)


def _legalize_multiwait(nc):
    """The walrus build here encodes exactly one semaphore wait per 64B
    instruction (NEURON_ISA_TPB_EVENTS has a single wait slot) and errors on
    Tile's multi-wait instructions.  Split surplus waits into standalone
    EVENT_SEMAPHORE instructions on the same engine, placed directly before
    the original instruction (same sequencer stream -> same semantics)."""
    n_ev = 0
    for f in nc.m.functions:
        for blk in f.blocks:
            insts = blk.instructions
            out = []
            changed = False
            for inst in list(insts):
                si = getattr(inst, "sync_info", None)
                waits = list(si.on_wait) if si is not None else []
                if len(waits) > 1:
                    changed = True
                    updates = list(si.on_update)
                    for w in waits[:-1]:
                        ev = mybir.InstEventSemaphore(
                            name=f"{inst.name}-sw{n_ev}", ins=[], outs=[])
                        n_ev += 1
                        ev.engine = inst.engine
                        ev.sync_info = mybir.SyncInfo(on_wait=[w], on_update=[])
                        out.append(ev)
                    inst.sync_info = mybir.SyncInfo(
                        on_wait=[waits[-1]], on_update=updates)
                out.append(inst)
            if changed:
                insts.clear()
                insts.extend(out)


def _build(ones_nw=False):
    nc = bass.Bass()
    # x^T slab layout per core: xt[tt, p, ib, t] = x[tt*128 + t, ib*128 + p], bf16
    xt_h = nc.dram_tensor("xt", [TB, 128, KB, 128], BF16, kind="ExternalInput")
    # weight layout: wt[p, ib, o] = W[o, ib*128 + p] with W = sum_l conv_w[l]/L, bf16
    wt_h = nc.dram_tensor("wt", [128, KB, H], BF16, kind="ExternalInput")
    nw_h = nc.dram_tensor("nw", [H], F32, kind="ExternalInput")
    out_h = nc.dram_tensor("out", [TPC, H], BF16, kind="ExternalOutput")

    with tile.TileContext(nc) as tc, ExitStack() as ctx:
        xpool = ctx.enter_context(tc.tile_pool(name="x", bufs=1))
        wpool = ctx.enter_context(tc.tile_pool(name="w", bufs=1))
        cpool = ctx.enter_context(tc.tile_pool(name="consts", bufs=1))
        # out DMA acks lag ~2-3us behind their data; 6 bufs keep the o_sb
        # WAR (tile n vs tile n-6's DMA completion) off the critical path.
        opool = ctx.enter_context(tc.tile_pool(name="out", bufs=6))
        spool = ctx.enter_context(tc.tile_pool(name="scratch", bufs=2))
        stats = ctx.enter_context(tc.tile_pool(name="stats", bufs=8))
        psum = ctx.enter_context(tc.tile_pool(name="psum", bufs=4, space="PSUM"))

        w_sb = wpool.tile([128, KB, H], BF16)
        x_sb = xpool.tile([128, TB, KB, 128], BF16)
        G = 2  # token blocks computed w-chunk-major while the weights stream

        def x_dma(eng, a, b, ib0=0, ib1=KB):
            # x slabs [a, b) x contraction blocks [ib0, ib1) -> x_sb slice
            src = bass.AP(tensor=xt_h, offset=a * 128 * XSL + ib0 * 128,
                          ap=[[XSL, 128], [128 * XSL, b - a],
                              [1, (ib1 - ib0) * 128]])
            eng.dma_start(out=x_sb[:, a:b, ib0:ib1, :], in_=src)

        # The 8 weight chunks (consumption order c1..c8 = oh-major, ib
        # pairs) are interleaved across BOTH HWDGE rings so their arrival
        # spacing is halved; x for the phase-1 blocks rides between them,
        # and the bulk x prefetch is queued BEHIND the weights so it
        # cannot compete with them for HBM bandwidth.
        def w_dma(eng, k, ib0, ib1):  # w chunk: oh half k, ib range
            oh = k
            eng.dma_start(
                out=w_sb[:, ib0:ib1, oh * 512:(oh + 1) * 512],
                in_=wt_h[:, ib0:ib1, oh * 512:(oh + 1) * 512])

        # sync ring: xA, c2, xB, c4, c6, c8, x8-11 (then output DMAs)
        x_dma(nc.sync, 0, G, 0, KB // 2)
        w_dma(nc.sync, 0, 2, 4)   # c2 = oh0 ib23
        x_dma(nc.sync, 0, G, KB // 2, KB)
        w_dma(nc.sync, 0, 6, 8)   # c4 = oh0 ib67
        w_dma(nc.sync, 1, 2, 4)   # c6 = oh1 ib23
        w_dma(nc.sync, 1, 6, 8)   # c8 = oh1 ib67
        x_dma(nc.sync, 3, 4)      # x3: lands right after c8, well before
        x_dma(nc.sync, 8, 12)     # tt3 needs it (was just-in-time behind
                                  # c5/c7 on the scalar ring)
        # scalar ring: c1 (split fine so the first matmul starts ~1us
        # earlier), c3, x2, c5, c7, x3, x4-7, x12-15
        w_dma(nc.scalar, 0, 0, 1)  # c1a = oh0 ib0
        w_dma(nc.scalar, 0, 1, 2)  # c1b = oh0 ib1
        w_dma(nc.scalar, 0, 4, 6)  # c3 = oh0 ib45
        x_dma(nc.scalar, 2, 3)     # x2 (feeds the phase-1b fillers)
        w_dma(nc.scalar, 1, 0, 2)  # c5 = oh1 ib01
        w_dma(nc.scalar, 1, 4, 6)  # c7 = oh1 ib45
        x_dma(nc.scalar, 4, 8)
        x_dma(nc.scalar, 12, 16)

        # norm_w broadcast rides the gpsimd (SWDGE) ring, off the two HWDGE
        # rings that carry the latency-critical w/x stream.
        nw_sb = cpool.tile([128, H], F32)
        nc.gpsimd.dma_start(
            out=nw_sb, in_=bass.AP(tensor=nw_h, offset=0, ap=[[0, 128], [1, H]]))

        zero_sb = cpool.tile([128, 1], F32)
        nc.vector.memset(zero_sb, 0.0)
        eps_sb = cpool.tile([128, 1], F32)
        nc.vector.memset(eps_sb, EPS)

        # PE warm-up: dummy matmuls bridge the gap until the first w/x
        # chunks land, so the HAM clock gate sees uninterrupted activity
        # from well before the first real matmul and ramps the PE to
        # 2.4 GHz ~3.4us after the stream starts.  They write regions of
        # psum that later blocks overwrite (start=True clears the bank), so
        # no extra PSUM bank is needed.
        dummy = cpool.tile([128, 128], BF16)
        nc.vector.memset(dummy, 0.0)
        yps = [psum.tile([128, H], F32, name="yp", tag="yp") for _ in range(G)]
        # tt2's psum tile doubles as the dummy-matmul target: its real
        # accumulation group only opens in phase 1b (start=True clears the
        # bank), so dummies can write it any time before that - unlike
        # yps[0/1], whose groups are OPEN during phase 1a (a dummy's
        # start=True there would clear live partial sums).
        yp2 = psum.tile([128, H], F32, name="yp", tag="yp")

        def warm(n):
            for _ in range(n):
                nc.tensor.matmul(yp2[:, 0:128], dummy, dummy,
                                 start=True, stop=True)

        warm(34)

        sq = spool.tile([128, H], BF16)  # squares scratch, shared (write-only)

        def mk_square(yp, oh, half_sums):
            sl = slice(oh * 512, (oh + 1) * 512)
            nc.scalar.activation(out=sq[:, sl], in_=yp[:, sl],
                                 func=AF.Square, bias=zero_sb,
                                 accum_out=half_sums[:, oh:oh + 1])

        def finish_norm(tt, yp, half_sums, last=False, ones_nw=False):
            ssum = stats.tile([128, 1], F32)
            nc.vector.tensor_add(out=ssum, in0=half_sums[:, 0:1],
                                 in1=half_sums[:, 1:2])
            # std = sqrt(mean + eps); rstd = 1/std
            std = stats.tile([128, 1], F32)
            nc.scalar.activation(out=std, in_=ssum, func=AF.Sqrt,
                                 bias=eps_sb, scale=1.0 / H)
            rstd = stats.tile([128, 1], F32)
            nc.vector.reciprocal(out=rstd, in_=std)
            # out = (y * rstd) * norm_w, written bf16.  The last block is
            # split finer so its DMA starts as soon as possible.
            if last and ones_nw:
                # norm_w is all-ones: split the final scale across BOTH
                # engines so the two halves run in parallel (ACT's free
                # per-partition scale does y*rstd directly), and the two
                # half DMAs across two rings so the triggers overlap.
                # TWO dedicated, never-reused half tiles: pooled tiles'
                # WAR deps run through 8 shared DMA semaphore lanes (false
                # serialization on an old DMA ack), and a single shared
                # tile WAW-serializes the two engines' writes.
                o_h0 = opool.tile([128, 512], BF16, name="o_last0",
                                  tag="olast0", bufs=1)
                o_h1 = opool.tile([128, 512], BF16, name="o_last1",
                                  tag="olast1", bufs=1)
                nc.scalar.activation(out=o_h1, in_=yp[:, 512:1024],
                                     func=AF.Copy, scale=rstd)
                nc.vector.scalar_tensor_tensor(
                    out=o_h0, in0=yp[:, 0:512], scalar=rstd,
                    in1=nw_sb[:, 0:512], op0=OP.mult, op1=OP.mult,
                )
                nc.sync.dma_start(out=out_h[tt * 128:(tt + 1) * 128, 0:512],
                                  in_=o_h0)
                nc.gpsimd.dma_start(
                    out=out_h[tt * 128:(tt + 1) * 128, 512:1024],
                    in_=o_h1)
                return
            if last:
                o_sb = opool.tile([128, H], BF16, name="o_last",
                                  tag="olast", bufs=1)
            else:
                o_sb = opool.tile([128, H], BF16, name="o_sb", tag="o")
            for q in range(2):
                sl = slice(q * 512, (q + 1) * 512)
                nc.vector.scalar_tensor_tensor(
                    out=o_sb[:, sl], in0=yp[:, sl], scalar=rstd,
                    in1=nw_sb[:, sl], op0=OP.mult, op1=OP.mult,
                )
                if last:
                    nc.sync.dma_start(
                        out=out_h[tt * 128:(tt + 1) * 128, sl],
                        in_=o_sb[:, sl])
            if not last:
                nc.sync.dma_start(out=out_h[tt * 128:(tt + 1) * 128, :],
                                  in_=o_sb)

        # Phase 1: w-chunk-major over the first G token blocks.  Each
        # arriving 256KB w chunk feeds 2*G matmuls, so the PE's weight
        # demand drops to what HBM can deliver while 8 cores all pull their
        # weights.  Dummy matmuls pad the arrival gaps so the HAM clock
        # gate sees continuous activity and ramps to 2.4 GHz early.
        hs = {}
        # Phase 1a: oh0 chunks c1..c4, chunk-major over t0,t1.  ib-major so
        # a late-arriving ib never head-of-line-blocks matmuls whose data
        # is already present; the c1a->c1b arrival gap (~2us, the fabric is
        # still ramping) is bridged with dummies so the clock gate holds.
        for ibp in range(0, KB, 2):
            for ib in (ibp, ibp + 1):
                for t in range(G):
                    nc.tensor.matmul(
                        yps[t][:, 0:512],
                        x_sb[:, t, ib, :],
                        w_sb[:, ib, 0:512],
                        start=(ib == 0),
                        stop=(ib == KB - 1),
                    )
                if ib == 0:
                    warm(20)
        for t in range(G):
            hs[t] = stats.tile([128, 2], F32, name="hs", tag="hs")
            mk_square(yps[t], 0, hs[t])
        # Phase 1b: oh1 chunks c5..c8 over t0,t1; the chunk-arrival gaps
        # are filled with REAL work: tt2's oh0 matmuls against the already
        # resident oh0 weight chunks (keeps the PE and its clock gate busy).
        f_ib = 0
        for ibp in range(0, KB, 2):
            for ib in (ibp, ibp + 1):
                for t in range(G):
                    nc.tensor.matmul(
                        yps[t][:, 512:1024],
                        x_sb[:, t, ib, :],
                        w_sb[:, ib, 512:1024],
                        start=(ib == 0),
                        stop=(ib == KB - 1),
                    )
            for _ in range(2):
                if ibp > 0 and f_ib < KB:
                    nc.tensor.matmul(
                        yp2[:, 0:512],
                        x_sb[:, 2, f_ib, :],
                        w_sb[:, f_ib, 0:512],
                        start=(f_ib == 0),
                        stop=(f_ib == KB - 1),
                    )
                    f_ib += 1
        for t in range(G):
            mk_square(yps[t], 1, hs[t])
        for t in range(G):
            finish_norm(t, yps[t], hs[t])

        # Phase 1c: finish tt2 (rest of oh0, then oh1).
        hs2 = stats.tile([128, 2], F32, name="hs", tag="hs")
        while f_ib < KB:
            nc.tensor.matmul(
                yp2[:, 0:512], x_sb[:, 2, f_ib, :], w_sb[:, f_ib, 0:512],
                start=(f_ib == 0), stop=(f_ib == KB - 1))
            f_ib += 1
        mk_square(yp2, 0, hs2)
        for ib in range(KB):
            nc.tensor.matmul(
                yp2[:, 512:1024], x_sb[:, 2, ib, :], w_sb[:, ib, 512:1024],
                start=(ib == 0), stop=(ib == KB - 1))
        mk_square(yp2, 1, hs2)
        finish_norm(2, yp2, hs2)

        # Phase 2: weights are resident; token-block-major.
        for tt in range(G + 1, TB):
            yp = psum.tile([128, H], F32, name="yp", tag="yp")
            half_sums = stats.tile([128, 2], F32, name="hs", tag="hs")
            for oh in range(NOH):
                for ib in range(KB):
                    nc.tensor.matmul(
                        yp[:, oh * 512:(oh + 1) * 512],
                        x_sb[:, tt, ib, :],
                        w_sb[:, ib, oh * 512:(oh + 1) * 512],
                        start=(ib == 0),
                        stop=(ib == KB - 1),
                    )
                mk_square(yp, oh, half_sums)
            finish_norm(tt, yp, half_sums, last=(tt == TB - 1),
                        ones_nw=ones_nw)

    _legalize_multiwait(nc)
    return nc


def host_prep(x, conv_w, norm_w):
    """Shard + lay out the full inputs into per-core device input maps."""
    bf16 = ml_dtypes.bfloat16

    # Collapse the 20 1x1 convs: W[o,i] = sum_l conv_w[l,o,i] / L
    w = np.asarray(conv_w).sum(axis=0) * (1.0 / L)          # [H(o), H(i)] f32
    # wt[p, ib, o] = W[o, ib*128+p]
    wt = np.ascontiguousarray(
        w.reshape(H, KB, 128).transpose(2, 1, 0).astype(bf16))
    nw = np.ascontiguousarray(np.asarray(norm_w), dtype=np.float32)

    x2d = np.asarray(x).reshape(TOK, H)
    xbf = x2d.astype(bf16)

    in_maps = []
    for c in range(N_CORES):
        xc = xbf[c * TPC:(c + 1) * TPC]                      # [TPC, H]
        # xt[tt, p, ib, t] = xc[tt*128+t, ib*128+p]
        xtc = np.ascontiguousarray(
            xc.reshape(TB, 128, KB, 128).transpose(0, 3, 2, 1))
        in_maps.append({"xt": xtc, "wt": wt, "nw": nw})
    return in_maps


def kernel(x, conv_w, norm_w):
    global _BUILT, LAST_RESULTS
    ones_nw = bool(np.allclose(np.asarray(norm_w), 1.0))
    if _BUILT is None:
        _BUILT = {}
    if ones_nw not in _BUILT:
        _BUILT[ones_nw] = _build(ones_nw)
    nc = _BUILT[ones_nw]

    x = np.asarray(x)
    out_dtype = x.dtype
    in_maps = host_prep(x, conv_w, norm_w)

    res = run_bass_kernel_spmd(nc, in_maps, core_ids=list(range(N_CORES)))
    LAST_RESULTS = res

    out = np.concatenate([r["out"] for r in res.results], axis=0)
    return out.reshape(B, S, H).astype(out_dtype, copy=False)


# revision 54
# speedup vs baseline: 1.0535x; 1.0204x over previous
"""Trainium2 Bass kernel for Conv2D(sum of 20 1x1 convs) + QwenRMSNorm.

Math: y = einsum("bsi,loi->bso", x, conv_w) / L ; out = rmsnorm(y) * norm_w.
Since x does not depend on l, the 20-matrix contraction collapses to a single
matmul with W = sum_l conv_w[l] / L.  Host pre-sums/transposes/casts the weight
(one [H,H] matrix) and lays out x as token-sharded, hidden-major bf16 slabs;
the 8 NeuronCores each run matmul (bf16, fp32 accum) + RMSNorm on their 2048
tokens.  All device compute is token-local; no collectives.

Scheduling notes:
  * Per-core HBM share is ~360 GB/s and both HWDGE rings (SP=sync,
    Act=scalar) split it, so the startup critical path is "first w chunk +
    first x chunk".  w streams in 8 fine chunks in exact consumption order
    on the scalar ring; the bulk x prefetch is queued BEHIND w on the same
    ring (FIFO) so it cannot steal bandwidth from the weight load.  Only
    x[tt0,tt1] + norm_w ride the sync ring early.
  * Output is bf16 (upcast on host): halves the output traffic and the
    end-of-kernel DMA drain.  Output DMAs ride the sync ring, which is idle
    after the first microseconds.
  * PE warm-up: the HAM clock gate keeps the PE at 1.2 GHz until it has been
    busy ~3.4us without gaps.  Dummy matmuls bridge the first-DMA wait.
  * x lives in one big SBUF tile; Tile's region tracker gives per-slice
    dependencies so matmuls only gate on the DMA stage that carries their
    slab.
  * While the weights stream (HBM-bound), the first blocks run
    w-chunk-major so each arriving chunk feeds several matmuls, and the
    chunk-arrival gaps in the oh1 half are filled with tt2's oh0 matmuls
    against already-resident chunks - the PE never idles long enough for
    the clock gate to drop.
"""

import numpy as np
import ml_dtypes
from contextlib import ExitStack

import concourse.bass as bass
import concourse.mybir as mybir
import concourse.tile as tile
from concourse.bass_utils import run_bass_kernel_spmd

N_CORES = 8
B, S, H, L = 4, 4096, 1024, 20
TOK = B * S               # 16384 tokens
TPC = TOK // N_CORES      # 2048 tokens per core
TB = TPC // 128           # 16 token-blocks of 128 per core
KB = H // 128             # 8 contraction blocks
NOH = H // 512            # 2 psum halves of the output row
XSL = KB * 128            # 1024 elements per (partition, token-block) of x
EPS = 1e-6

BF16 = mybir.dt.bfloat16
F32 = mybir.dt.float32
AF = mybir.ActivationFunctionType
OP = mybir.AluOpType

_BUILT = None       # cached Bass program
LAST_RESULTS = None  # BassKernelResults of the most recent run (for test harness)


def _legalize_multiwait(nc):
    """The walrus build here encodes exactly one semaphore wait per 64B
    instruction (NEURON_ISA_TPB_EVENTS has a single wait slot) and errors on
    Tile's multi-wait instructions.  Split surplus waits into standalone
    EVENT_SEMAPHORE instructions on the same engine, placed directly before
    the original instruction (same sequencer stream -> same semantics)."""
    n_ev = 0
    for f in nc.m.functions:
        for blk in f.blocks:
            insts = blk.instructions
            out = []
            changed = False
            for inst in list(insts):
                si = getattr(inst, "sync_info", None)
                waits = list(si.on_wait) if si is not None else []
                if len(waits) > 1:
                    changed = True
                    updates = list(si.on_update)
                    for w in waits[:-1]:
                        ev = mybir.InstEventSemaphore(
                            name=f"{inst.name}-sw{n_ev}", ins=[], outs=[])
                        n_ev += 1
                        ev.engine = inst.engine
                        ev.sync_info = mybir.SyncInfo(on_wait=[w], on_update=[])
                        out.append(ev)
                    inst.sync_info = mybir.SyncInfo(
                        on_wait=[waits[-1]], on_update=updates)
                out.append(inst)
            if changed:
                insts.clear()
                insts.extend(out)


def _build(ones_nw=False):
    nc = bass.Bass()
    # x^T slab layout per core: xt[tt, p, ib, t] = x[tt*128 + t, ib*128 + p], bf16
    xt_h = nc.dram_tensor("xt", [TB, 128, KB, 128], BF16, kind="ExternalInput")
    # weight layout: wt[p, ib, o] = W[o, ib*128 + p] with W = sum_l conv_w[l]/L, bf16
    wt_h = nc.dram_tensor("wt", [128, KB, H], BF16, kind="ExternalInput")
    nw_h = nc.dram_tensor("nw", [H], F32, kind="ExternalInput")
    out_h = nc.dram_tensor("out", [TPC, H], BF16, kind="ExternalOutput")

    with tile.TileContext(nc) as tc, ExitStack() as ctx:
        xpool = ctx.enter_context(tc.tile_pool(name="x", bufs=1))
        wpool = ctx.enter_context(tc.tile_pool(name="w", bufs=1))
        cpool = ctx.enter_context(tc.tile_pool(name="consts", bufs=1))
        # out DMA acks lag ~2-3us behind their data; 6 bufs keep the o_sb
        # WAR (tile n vs tile n-6's DMA completion) off the critical path.
        opool = ctx.enter_context(tc.tile_pool(name="out", bufs=6))
        spool = ctx.enter_context(tc.tile_pool(name="scratch", bufs=2))
        stats = ctx.enter_context(tc.tile_pool(name="stats", bufs=8))
        psum = ctx.enter_context(tc.tile_pool(name="psum", bufs=4, space="PSUM"))

        w_sb = wpool.tile([128, KB, H], BF16)
        x_sb = xpool.tile([128, TB, KB, 128], BF16)
        G = 2  # token blocks computed w-chunk-major while the weights stream

        def x_dma(eng, a, b, ib0=0, ib1=KB):
            # x slabs [a, b) x contraction blocks [ib0, ib1) -> x_sb slice
            src = bass.AP(tensor=xt_h, offset=a * 128 * XSL + ib0 * 128,
                          ap=[[XSL, 128], [128 * XSL, b - a],
                              [1, (ib1 - ib0) * 128]])
            eng.dma_start(out=x_sb[:, a:b, ib0:ib1, :], in_=src)

        # The 8 weight chunks (consumption order c1..c8 = oh-major, ib
        # pairs) are interleaved across BOTH HWDGE rings so their arrival
        # spacing is halved; x for the phase-1 blocks rides between them,
        # and the bulk x prefetch is queued BEHIND the weights so it
        # cannot compete with them for HBM bandwidth.
        def w_dma(eng, k, ib0, ib1):  # w chunk: oh half k, ib range
            oh = k
            eng.dma_start(
                out=w_sb[:, ib0:ib1, oh * 512:(oh + 1) * 512],
                in_=wt_h[:, ib0:ib1, oh * 512:(oh + 1) * 512])

        # sync ring: xA, c2, xB, c4, c6, c8, x8-11 (then output DMAs)
        x_dma(nc.sync, 0, G, 0, KB // 2)
        w_dma(nc.sync, 0, 2, 4)   # c2 = oh0 ib23
        x_dma(nc.sync, 0, G, KB // 2, KB)
        w_dma(nc.sync, 0, 6, 8)   # c4 = oh0 ib67
        w_dma(nc.sync, 1, 2, 4)   # c6 = oh1 ib23
        w_dma(nc.sync, 1, 6, 8)   # c8 = oh1 ib67
        x_dma(nc.sync, 8, 12)
        # scalar ring: c1 (split fine so the first matmul starts ~1us
        # earlier), c3, x2, c5, c7, x3, x4-7, x12-15
        w_dma(nc.scalar, 0, 0, 1)  # c1a = oh0 ib0
        w_dma(nc.scalar, 0, 1, 2)  # c1b = oh0 ib1
        w_dma(nc.scalar, 0, 4, 6)  # c3 = oh0 ib45
        x_dma(nc.scalar, 2, 3)     # x2 (feeds the phase-1b fillers)
        w_dma(nc.scalar, 1, 0, 2)  # c5 = oh1 ib01
        w_dma(nc.scalar, 1, 4, 6)  # c7 = oh1 ib45
        x_dma(nc.scalar, 3, 4)     # x3
        x_dma(nc.scalar, 4, 8)
        x_dma(nc.scalar, 12, 16)

        # norm_w broadcast rides the gpsimd (SWDGE) ring, off the two HWDGE
        # rings that carry the latency-critical w/x stream.
        nw_sb = cpool.tile([128, H], F32)
        nc.gpsimd.dma_start(
            out=nw_sb, in_=bass.AP(tensor=nw_h, offset=0, ap=[[0, 128], [1, H]]))

        zero_sb = cpool.tile([128, 1], F32)
        nc.vector.memset(zero_sb, 0.0)
        eps_sb = cpool.tile([128, 1], F32)
        nc.vector.memset(eps_sb, EPS)

        # PE warm-up: dummy matmuls bridge the gap until the first w/x
        # chunks land, so the HAM clock gate sees uninterrupted activity
        # from well before the first real matmul and ramps the PE to
        # 2.4 GHz ~3.4us after the stream starts.  They write regions of
        # psum that later blocks overwrite (start=True clears the bank), so
        # no extra PSUM bank is needed.
        dummy = cpool.tile([128, 128], BF16)
        nc.vector.memset(dummy, 0.0)
        yps = [psum.tile([128, H], F32, name="yp", tag="yp") for _ in range(G)]
        # tt2's psum tile doubles as the dummy-matmul target: its real
        # accumulation group only opens in phase 1b (start=True clears the
        # bank), so dummies can write it any time before that - unlike
        # yps[0/1], whose groups are OPEN during phase 1a (a dummy's
        # start=True there would clear live partial sums).
        yp2 = psum.tile([128, H], F32, name="yp", tag="yp")

        def warm(n):
            for _ in range(n):
                nc.tensor.matmul(yp2[:, 0:128], dummy, dummy,
                                 start=True, stop=True)

        warm(34)

        sq = spool.tile([128, H], BF16)  # squares scratch, shared (write-only)

        def mk_square(yp, oh, half_sums):
            sl = slice(oh * 512, (oh + 1) * 512)
            nc.scalar.activation(out=sq[:, sl], in_=yp[:, sl],
                                 func=AF.Square, bias=zero_sb,
                                 accum_out=half_sums[:, oh:oh + 1])

        def finish_norm(tt, yp, half_sums, last=False, ones_nw=False):
            ssum = stats.tile([128, 1], F32)
            nc.vector.tensor_add(out=ssum, in0=half_sums[:, 0:1],
                                 in1=half_sums[:, 1:2])
            # std = sqrt(mean + eps); rstd = 1/std
            std = stats.tile([128, 1], F32)
            nc.scalar.activation(out=std, in_=ssum, func=AF.Sqrt,
                                 bias=eps_sb, scale=1.0 / H)
            rstd = stats.tile([128, 1], F32)
            nc.vector.reciprocal(out=rstd, in_=std)
            # out = (y * rstd) * norm_w, written bf16.  The last block is
            # split finer so its DMA starts as soon as possible.
            if last and ones_nw:
                # norm_w is all-ones: split the final scale across BOTH
                # engines so the two halves run in parallel (ACT's free
                # per-partition scale does y*rstd directly), and the two
                # half DMAs across two rings so the triggers overlap.
                # TWO dedicated, never-reused half tiles: pooled tiles'
                # WAR deps run through 8 shared DMA semaphore lanes (false
                # serialization on an old DMA ack), and a single shared
                # tile WAW-serializes the two engines' writes.
                o_h0 = opool.tile([128, 512], BF16, name="o_last0",
                                  tag="olast0", bufs=1)
                o_h1 = opool.tile([128, 512], BF16, name="o_last1",
                                  tag="olast1", bufs=1)
                nc.scalar.activation(out=o_h1, in_=yp[:, 512:1024],
                                     func=AF.Copy, scale=rstd)
                nc.vector.scalar_tensor_tensor(
                    out=o_h0, in0=yp[:, 0:512], scalar=rstd,
                    in1=nw_sb[:, 0:512], op0=OP.mult, op1=OP.mult,
                )
                nc.sync.dma_start(out=out_h[tt * 128:(tt + 1) * 128, 0:512],
                                  in_=o_h0)
                nc.gpsimd.dma_start(
                    out=out_h[tt * 128:(tt + 1) * 128, 512:1024],
                    in_=o_h1)
                return
            if last:
                o_sb = opool.tile([128, H], BF16, name="o_last",
                                  tag="olast", bufs=1)
            else:
                o_sb = opool.tile([128, H], BF16, name="o_sb", tag="o")
            for q in range(2):
                sl = slice(q * 512, (q + 1) * 512)
                nc.vector.scalar_tensor_tensor(
                    out=o_sb[:, sl], in0=yp[:, sl], scalar=rstd,
                    in1=nw_sb[:, sl], op0=OP.mult, op1=OP.mult,
                )
                if last:
                    nc.sync.dma_start(
                        out=out_h[tt * 128:(tt + 1) * 128, sl],
                        in_=o_sb[:, sl])
            if not last:
                nc.sync.dma_start(out=out_h[tt * 128:(tt + 1) * 128, :],
                                  in_=o_sb)

        # Phase 1: w-chunk-major over the first G token blocks.  Each
        # arriving 256KB w chunk feeds 2*G matmuls, so the PE's weight
        # demand drops to what HBM can deliver while 8 cores all pull their
        # weights.  Dummy matmuls pad the arrival gaps so the HAM clock
        # gate sees continuous activity and ramps to 2.4 GHz early.
        hs = {}
        # Phase 1a: oh0 chunks c1..c4, chunk-major over t0,t1.  ib-major so
        # a late-arriving ib never head-of-line-blocks matmuls whose data
        # is already present; the c1a->c1b arrival gap (~2us, the fabric is
        # still ramping) is bridged with dummies so the clock gate holds.
        for ibp in range(0, KB, 2):
            for ib in (ibp, ibp + 1):
                for t in range(G):
                    nc.tensor.matmul(
                        yps[t][:, 0:512],
                        x_sb[:, t, ib, :],
                        w_sb[:, ib, 0:512],
                        start=(ib == 0),
                        stop=(ib == KB - 1),
                    )
                if ib == 0:
                    warm(20)
        for t in range(G):
            hs[t] = stats.tile([128, 2], F32, name="hs", tag="hs")
            mk_square(yps[t], 0, hs[t])
        # Phase 1b: oh1 chunks c5..c8 over t0,t1; the chunk-arrival gaps
        # are filled with REAL work: tt2's oh0 matmuls against the already
        # resident oh0 weight chunks (keeps the PE and its clock gate busy).
        f_ib = 0
        for ibp in range(0, KB, 2):
            for ib in (ibp, ibp + 1):
                for t in range(G):
                    nc.tensor.matmul(
                        yps[t][:, 512:1024],
                        x_sb[:, t, ib, :],
                        w_sb[:, ib, 512:1024],
                        start=(ib == 0),
                        stop=(ib == KB - 1),
                    )
            for _ in range(2):
                if ibp > 0 and f_ib < KB:
                    nc.tensor.matmul(
                        yp2[:, 0:512],
                        x_sb[:, 2, f_ib, :],
                        w_sb[:, f_ib, 0:512],
                        start=(f_ib == 0),
                        stop=(f_ib == KB - 1),
                    )
                    f_ib += 1
        for t in range(G):
            mk_square(yps[t], 1, hs[t])
        for t in range(G):
            finish_norm(t, yps[t], hs[t])

        # Phase 1c: finish tt2 (rest of oh0, then oh1).
        hs2 = stats.tile([128, 2], F32, name="hs", tag="hs")
        while f_ib < KB:
            nc.tensor.matmul(
                yp2[:, 0:512], x_sb[:, 2, f_ib, :], w_sb[:, f_ib, 0:512],
                start=(f_ib == 0), stop=(f_ib == KB - 1))
            f_ib += 1
        mk_square(yp2, 0, hs2)
        for ib in range(KB):
            nc.tensor.matmul(
                yp2[:, 512:1024], x_sb[:, 2, ib, :], w_sb[:, ib, 512:1024],
                start=(ib == 0), stop=(ib == KB - 1))
        mk_square(yp2, 1, hs2)
        finish_norm(2, yp2, hs2)

        # Phase 2: weights are resident; token-block-major.
        for tt in range(G + 1, TB):
            yp = psum.tile([128, H], F32, name="yp", tag="yp")
            half_sums = stats.tile([128, 2], F32, name="hs", tag="hs")
            for oh in range(NOH):
                for ib in range(KB):
                    nc.tensor.matmul(
                        yp[:, oh * 512:(oh + 1) * 512],
                        x_sb[:, tt, ib, :],
                        w_sb[:, ib, oh * 512:(oh + 1) * 512],
                        start=(ib == 0),
                        stop=(ib == KB - 1),
                    )
                mk_square(yp, oh, half_sums)
            finish_norm(tt, yp, half_sums, last=(tt == TB - 1),
                        ones_nw=ones_nw)

    _legalize_multiwait(nc)
    return nc


def host_prep(x, conv_w, norm_w):
    """Shard + lay out the full inputs into per-core device input maps."""
    bf16 = ml_dtypes.bfloat16

    # Collapse the 20 1x1 convs: W[o,i] = sum_l conv_w[l,o,i] / L
    w = np.asarray(conv_w).sum(axis=0) * (1.0 / L)          # [H(o), H(i)] f32
    # wt[p, ib, o] = W[o, ib*128+p]
    wt = np.ascontiguousarray(
        w.reshape(H, KB, 128).transpose(2, 1, 0).astype(bf16))
    nw = np.ascontiguousarray(np.asarray(norm_w), dtype=np.float32)

    x2d = np.asarray(x).reshape(TOK, H)
    xbf = x2d.astype(bf16)

    in_maps = []
    for c in range(N_CORES):
        xc = xbf[c * TPC:(c + 1) * TPC]                      # [TPC, H]
        # xt[tt, p, ib, t] = xc[tt*128+t, ib*128+p]
        xtc = np.ascontiguousarray(
            xc.reshape(TB, 128, KB, 128).transpose(0, 3, 2, 1))
        in_maps.append({"xt": xtc, "wt": wt, "nw": nw})
    return in_maps


def kernel(x, conv_w, norm_w):
    global _BUILT, LAST_RESULTS
    ones_nw = bool(np.allclose(np.asarray(norm_w), 1.0))
    if _BUILT is None:
        _BUILT = {}
    if ones_nw not in _BUILT:
        _BUILT[ones_nw] = _build(ones_nw)
    nc = _BUILT[ones_nw]

    x = np.asarray(x)
    out_dtype = x.dtype
    in_maps = host_prep(x, conv_w, norm_w)

    res = run_bass_kernel_spmd(nc, in_maps, core_ids=list(range(N_CORES)))
    LAST_RESULTS = res

    out = np.concatenate([r["out"] for r in res.results], axis=0)
    return out.reshape(B, S, H).astype(out_dtype, copy=False)
